# revision 7
# baseline (speedup 1.0000x reference)
"""CrossAttention kernel for 8 TRN2 NeuronCores (Bass/Tile, SPMD).

Sharding: core c handles batch b = c//2 and query-pixel half nh = c%2
(2048 of the 4096 spatial positions). k/v come from the full cond[b]
computed locally on each core, so no collectives are needed and each
core writes a disjoint (256, 2048) slice of the output.

Math per core (channels-major layout, C on partitions, N on free dim):
  gn_x   = GroupNorm(x[b])            (stats over full N, applied on half)
  gn_kv  = GroupNorm(cond[b])
  q      = wq @ gn_x[:, half] + bq    (256, 2048)
  k      = wk @ gn_kv + bk            (256, 4096)
  vT_aug = (gn_kv^T @ wv^T + bv) with a ones column per head  (4096, 4*65)
  per head h, n-chunk of 512:
    St   = scores^T in PSUM (m on partitions): lhsT = k-tile, rhs = q-chunk
    P^T  = exp(scale * St)            (ScalarE, PSUM->SBUF, 3 banks/instr)
    AV   = vT_aug_h as lhsT, P^T as rhs, accumulated over 32 m-tiles
           -> (65, 512) where row 64 is the softmax denominator
    O    = AV[0:64] * recip(AV[64]) broadcast (K=1 matmul broadcast + DVE)
  y      = wo @ O + bo + x[b][:, half]

No softmax max-subtraction: scores are ~N(0,1) after the 1/8 scale, and
exp is computed in f32 (overflow needs |s| > 88; verified empirically).
Big matmuls run in float32r (1 cyc/row at >=256 moving rows vs 4 for
f32). The BIR verifier requires f32r matmul operands to be *produced*
as f32r, so weight DRAM tensors are declared f32r (host pre-rounds to
tf32) and on-chip operands are written as f32r by DVE/ACT ops.
"""

import os
import sys

import numpy as np

sys.path.insert(0, "/opt/trn_rl_repo")

import concourse.bass as bass  # noqa: E402
import concourse.mybir as mybir  # noqa: E402
import concourse.tile as tile  # noqa: E402
from concourse import bacc  # noqa: E402
from concourse.bass_utils import run_bass_kernel_spmd  # noqa: E402

B, C, H, W = 4, 256, 64, 64
N = H * W  # 4096
NH = 4
HD = C // NH  # 64
GROUPS = 32
CPG = C // GROUPS  # 8 channels per group
EPS = 1e-5
NCORES = 8
NHALF = N // 2  # 2048
NCH = NHALF // 512  # 4 n-chunks of 512
MT = N // 128  # 32 m-tiles
GSZ = 3  # m-tiles per exp group (3 PSUM banks per St tile)
F32 = mybir.dt.float32
F32R = mybir.dt.float32r
AX = mybir.AxisListType
OP = mybir.AluOpType
AF = mybir.ActivationFunctionType
SCALE = HD ** -0.5

_cache = {}


def _build():
    # KV bisection levels: 1=GN+convs only, 2=+St/exp, 3=+AV+normalize,
    # 4=full (default)
    KV = int(os.environ.get("KV", "4"))
    nc = bacc.Bacc("TRN2", target_bir_lowering=False, debug=False,
                   enable_asserts=False, num_devices=NCORES)

    d_x = nc.dram_tensor("xb", [2, 128, N], F32, kind="ExternalInput")
    d_c = nc.dram_tensor("condb", [2, 128, N], F32, kind="ExternalInput")
    d_wq = nc.dram_tensor("wqT", [2, 128, 256], F32R, kind="ExternalInput")
    d_wk = nc.dram_tensor("wkT", [2, 128, 256], F32R, kind="ExternalInput")
    d_wv = nc.dram_tensor("wvTa", [2, 128, 260], F32R, kind="ExternalInput")
    d_wo = nc.dram_tensor("woT", [2, 128, 256], F32R, kind="ExternalInput")
    d_bva = nc.dram_tensor("bva", [1, 260], F32R, kind="ExternalInput")
    d_bq = nc.dram_tensor("bq2", [128, 2], F32, kind="ExternalInput")
    d_bk = nc.dram_tensor("bk2", [128, 2], F32, kind="ExternalInput")
    d_bo = nc.dram_tensor("bo2", [128, 2], F32, kind="ExternalInput")
    d_gqA = nc.dram_tensor("gqA", [128, 2], F32, kind="ExternalInput")
    d_gqB = nc.dram_tensor("gqB", [128, 2], F32, kind="ExternalInput")
    d_gkA = nc.dram_tensor("gkA", [128, 2], F32, kind="ExternalInput")
    d_gkB = nc.dram_tensor("gkB", [128, 2], F32, kind="ExternalInput")
    d_gm = nc.dram_tensor("gmap", [128, 16], F32, kind="ExternalInput")
    d_gmb = nc.dram_tensor("gmapb", [16, 128], F32, kind="ExternalInput")
    d_o64 = nc.dram_tensor("ones64", [1, 64], F32, kind="ExternalInput")
    d_o128 = nc.dram_tensor("ones128", [1, 128], F32R, kind="ExternalInput")
    d_out = nc.dram_tensor("out", [2, 128, NHALF], F32, kind="ExternalOutput")

    PS = bass.MemorySpace.PSUM

    with tile.TileContext(nc) as tc:
        with (
            tc.tile_pool(name="persist", bufs=1) as pp,
            tc.tile_pool(name="stps", bufs=2, space=PS) as stps,
            tc.tile_pool(name="smps", bufs=2, space=PS) as smps,
        ):
            # ---- persistent tensors (consts, q, k, vT) ----
            wq_sb = pp.tile([128, 2 * 256], F32R)
            wk_sb = pp.tile([128, 2 * 256], F32R)
            wv_sb = pp.tile([128, 2 * 260], F32R)
            wo_sb = pp.tile([128, 2 * 256], F32R)
            bva_sb = pp.tile([1, 260], F32R)
            bq_sb = pp.tile([128, 2], F32)
            bk_sb = pp.tile([128, 2], F32)
            bo_sb = pp.tile([128, 2], F32)
            gqA_sb = pp.tile([128, 2], F32)
            gqB_sb = pp.tile([128, 2], F32)
            gkA_sb = pp.tile([128, 2], F32)
            gkB_sb = pp.tile([128, 2], F32)
            gm_sb = pp.tile([128, 16], F32)
            gmb_sb = pp.tile([16, 128], F32)
            o64_sb = pp.tile([1, 64], F32)
            o128_sb = pp.tile([1, 128], F32R)

            for blk in range(2):
                nc.sync.dma_start(wq_sb[:, blk * 256:(blk + 1) * 256], d_wq[blk])
                nc.sync.dma_start(wk_sb[:, blk * 256:(blk + 1) * 256], d_wk[blk])
                nc.sync.dma_start(wv_sb[:, blk * 260:(blk + 1) * 260], d_wv[blk])
                nc.sync.dma_start(wo_sb[:, blk * 256:(blk + 1) * 256], d_wo[blk])
            nc.sync.dma_start(bva_sb[:], d_bva[:])
            nc.sync.dma_start(bq_sb[:], d_bq[:])
            nc.sync.dma_start(bk_sb[:], d_bk[:])
            nc.sync.dma_start(bo_sb[:], d_bo[:])
            nc.sync.dma_start(gqA_sb[:], d_gqA[:])
            nc.sync.dma_start(gqB_sb[:], d_gqB[:])
            nc.sync.dma_start(gkA_sb[:], d_gkA[:])
            nc.sync.dma_start(gkB_sb[:], d_gkB[:])
            nc.sync.dma_start(gm_sb[:], d_gm[:])
            nc.sync.dma_start(gmb_sb[:], d_gmb[:])
            nc.sync.dma_start(o64_sb[:], d_o64[:])
            nc.sync.dma_start(o128_sb[:], d_o128[:])

            q_sb = [pp.tile([128, NHALF], F32R, name=f"q{m}") for m in range(2)]
            k_sb = [pp.tile([128, N], F32R, name=f"k{m}") for m in range(2)]
            vT_sb = pp.tile([128, MT * 260], F32R)

            # ---- phase 1: GroupNorm + q/k/v convs ----
            with tc.tile_pool(name="ph1", bufs=1) as p1:
                scr = p1.tile([128, N], F32)

                def group_norm(t_raw, xn, gA, gB, apply_cols):
                    """Stats over full N, apply scale/shift to xn (f32r)."""
                    for blk in range(2):
                        st = p1.tile([128, 2], F32, name="st", tag="st")
                        nc.vector.reduce_sum(st[:, 0:1], t_raw[blk][:], axis=AX.X)
                        nc.scalar.activation(scr[:], t_raw[blk][:], AF.Square,
                                             accum_out=st[:, 1:2])
                        psg = smps.tile([128, 512], F32, name="psg", tag="sm")
                        nc.tensor.matmul(psg[0:16, 0:2], gm_sb[:], st[:],
                                         start=True, stop=True)
                        g16 = p1.tile([16, 8], F32, name="g16", tag="g16")
                        # cols: 0 mean, 1 E[x^2], 2 mean^2, 3 var+eps,
                        #       4 std, 5 rstd, 6 mean, 7 rstd
                        nc.vector.tensor_scalar_mul(g16[:, 0:2], psg[0:16, 0:2],
                                                    1.0 / (CPG * N))
                        nc.vector.tensor_mul(g16[:, 2:3], g16[:, 0:1],
                                             g16[:, 0:1])
                        nc.vector.tensor_sub(g16[:, 3:4], g16[:, 1:2],
                                             g16[:, 2:3])
                        nc.vector.tensor_scalar_add(g16[:, 3:4], g16[:, 3:4],
                                                    EPS)
                        nc.scalar.activation(g16[:, 4:5], g16[:, 3:4], AF.Sqrt)
                        nc.vector.reciprocal(g16[:, 5:6], g16[:, 4:5])
                        nc.vector.tensor_copy(g16[:, 6:7], g16[:, 0:1])
                        nc.vector.tensor_copy(g16[:, 7:8], g16[:, 5:6])
                        psb = smps.tile([128, 512], F32, name="psb", tag="sm")
                        nc.tensor.matmul(psb[:, 0:2], gmb_sb[:], g16[:, 6:8],
                                         start=True, stop=True)
                        ab = p1.tile([128, 2], F32, name="ab", tag="ab")
                        # A = rstd * gamma ; Bb = beta - mean * A
                        nc.vector.tensor_mul(ab[:, 0:1], psb[:, 1:2],
                                             gA[:, blk:blk + 1])
                        nc.vector.tensor_mul(ab[:, 1:2], psb[:, 0:1],
                                             ab[:, 0:1])
                        nc.vector.tensor_sub(ab[:, 1:2], gB[:, blk:blk + 1],
                                             ab[:, 1:2])
                        nc.vector.tensor_scalar(
                            out=xn[blk][:, 0:apply_cols],
                            in0=t_raw[blk][:, 0:apply_cols],
                            scalar1=ab[:, 0:1], scalar2=ab[:, 1:2],
                            op0=OP.mult, op1=OP.add)

                def conv(w_sb, src, b_sb, dst, cols):
                    """dst[mb] = w^T-block @ src + bias, over `cols` cols."""
                    for mb in range(2):
                        for ch in range(cols // 512):
                            ps = stps.tile([128, 512 * GSZ], F32, name="ps",
                                           tag="st")
                            for kb in range(2):
                                nc.tensor.matmul(
                                    ps[:, 0:512],
                                    w_sb[:, kb * 256 + mb * 128:
                                         kb * 256 + mb * 128 + 128],
                                    src[kb][:, ch * 512:(ch + 1) * 512],
                                    start=(kb == 0), stop=(kb == 1))
                            nc.vector.tensor_scalar_add(
                                dst[mb][:, ch * 512:(ch + 1) * 512],
                                ps[:, 0:512], b_sb[:, mb:mb + 1])

                # PH1 bisection: dma < gn < q < k < v (default v = all)
                PH1 = os.environ.get("PH1", "v")
                lvl = ["dma", "gn", "q", "k", "v"].index(PH1)

                # x first: stats -> apply(half) -> q conv, then cond reuses
                # the same tile slots (tags r0/r1, n0/n1).
                x_t = [p1.tile([128, N], F32, name=f"x{m}", tag=f"r{m}")
                       for m in range(2)]
                for blk in range(2):
                    nc.sync.dma_start(x_t[blk][:], d_x[blk])
                xn_x = [p1.tile([128, N], F32R, name=f"nx{m}", tag=f"n{m}")
                        for m in range(2)]
                if lvl >= 1:
                    group_norm(x_t, xn_x, gqA_sb, gqB_sb, NHALF)
                if lvl >= 2:
                    conv(wq_sb, xn_x, bq_sb, q_sb, NHALF)

                c_t = [p1.tile([128, N], F32, name=f"c{m}", tag=f"r{m}")
                       for m in range(2)]
                for blk in range(2):
                    nc.sync.dma_start(c_t[blk][:], d_c[blk])
                xn_c = [p1.tile([128, N], F32R, name=f"nc{m}", tag=f"n{m}")
                        for m in range(2)]
                if lvl >= 1:
                    group_norm(c_t, xn_c, gkA_sb, gkB_sb, N)
                if lvl >= 3:
                    conv(wk_sb, xn_c, bk_sb, k_sb, N)

                # vT conv: for each 128-row m-tile, vT_aug tile (128, 260)
                for mt in range(MT if lvl >= 4 else 0):
                    ps = smps.tile([128, 512], F32, name="ps", tag="sm")
                    for kb in range(2):
                        nc.tensor.matmul(
                            ps[:, 0:260],
                            xn_c[kb][:, mt * 128:(mt + 1) * 128],
                            wv_sb[:, kb * 260:(kb + 1) * 260],
                            start=(kb == 0), stop=False)
                    nc.tensor.matmul(ps[:, 0:260], o128_sb[:], bva_sb[:],
                                     start=False, stop=True)
                    nc.vector.tensor_copy(
                        vT_sb[:, mt * 260:(mt + 1) * 260], ps[:, 0:260])

            # ---- phase 2: attention + output ----
            with tc.tile_pool(name="attn", bufs=1) as pa, \
                 tc.tile_pool(name="ptp", bufs=3) as ptp:
                O_sb = [pa.tile([128, NHALF], F32R, name=f"O{m}")
                        for m in range(2)]
                xh_t = [pa.tile([128, NHALF], F32, name=f"xh{m}")
                        for m in range(2)]
                for blk in range(2):
                    nc.sync.dma_start(xh_t[blk][:], d_x[blk][:, 0:NHALF])

                groups = [(s, min(s + GSZ, MT)) for s in range(0, MT, GSZ)]

                for h in range(NH if KV >= 2 else 0):
                    kb = h // 2
                    po = 64 * (h % 2)
                    for ch in range(NCH):
                        av = smps.tile([128, 512], F32, name="av", tag="sm")
                        qc = q_sb[kb][po:po + 64, ch * 512:(ch + 1) * 512]
                        for (g0, g1) in groups:
                            gs = g1 - g0
                            ps = stps.tile([128, 512 * GSZ], F32, name="ps",
                                           tag="st")
                            for j in range(gs):
                                mt = g0 + j
                                nc.tensor.matmul(
                                    ps[:, j * 512:(j + 1) * 512],
                                    k_sb[kb][po:po + 64,
                                             mt * 128:(mt + 1) * 128],
                                    qc,
                                    start=True, stop=True)
                            pt = ptp.tile([128, 512 * GSZ], F32R, name="pt",
                                          tag="pt")
                            nc.scalar.activation(pt[:, 0:gs * 512],
                                                 ps[:, 0:gs * 512],
                                                 AF.Exp, scale=SCALE)
                            if KV < 3:
                                continue
                            for j in range(gs):
                                mt = g0 + j
                                nc.tensor.matmul(
                                    av[0:65, :],
                                    vT_sb[:, mt * 260 + h * 65:
                                          mt * 260 + h * 65 + 65],
                                    pt[:, j * 512:(j + 1) * 512],
                                    start=(mt == 0), stop=(mt == MT - 1))
                        if KV < 3:
                            continue
                        # normalize: O = av[0:64] / av[64]
                        dn = pa.tile([1, 512], F32, name="dn", tag="dn")
                        nc.vector.reciprocal(dn[:], av[64:65, :])
                        pb = smps.tile([128, 512], F32, name="pb", tag="sm")
                        nc.tensor.matmul(pb[0:64, :], o64_sb[:], dn[:],
                                         start=True, stop=True)
                        bc = pa.tile([64, 512], F32, name="bc", tag="bc")
                        nc.vector.tensor_copy(bc[:], pb[0:64, :])
                        nc.vector.tensor_mul(
                            O_sb[kb][po:po + 64, ch * 512:(ch + 1) * 512],
                            av[0:64, :], bc[:])

                # y = wo @ O + bo + x_half
                for mb in range(2 if KV >= 4 else 0):
                    for ch in range(NCH):
                        ps = smps.tile([128, 512], F32, name="ps", tag="sm")
                        for kb in range(2):
                            nc.tensor.matmul(
                                ps[:, 0:512],
                                wo_sb[:, kb * 256 + mb * 128:
                                      kb * 256 + mb * 128 + 128],
                                O_sb[kb][:, ch * 512:(ch + 1) * 512],
                                start=(kb == 0), stop=(kb == 1))
                        yt = pa.tile([128, 512], F32, name="yt", tag="yt")
                        nc.vector.tensor_scalar_add(yt[:], ps[:, 0:512],
                                                    bo_sb[:, mb:mb + 1])
                        nc.vector.tensor_add(
                            yt[:], yt[:],
                            xh_t[mb][:, ch * 512:(ch + 1) * 512])
                        nc.sync.dma_start(
                            d_out[mb][:, ch * 512:(ch + 1) * 512], yt[:])

    nc.compile()
    return nc


def _tf32(a):
    """Round f32 -> tf32-style (13 mantissa LSBs dropped, round nearest)."""
    u = a.astype(np.float32).view(np.uint32).copy()
    lsb = (u >> 13) & 1
    u = (u + 0x0FFF + lsb) & np.uint32(0xFFFFE000)
    return u.view(np.float32)


def _prep_inputs(x, cond, gn_q_w, gn_q_b, gn_kv_w, gn_kv_b,
                 wq, bq, wk, bk, wv, bv, wo, bo):
    """Build the 8 per-core input maps (all host-side slicing/layout)."""
    f = np.float32
    x = np.asarray(x, f)
    cond = np.asarray(cond, f)
    wq, bq = np.asarray(wq, f), np.asarray(bq, f)
    wk, bk = np.asarray(wk, f), np.asarray(bk, f)
    wv, bv = np.asarray(wv, f), np.asarray(bv, f)
    wo, bo = np.asarray(wo, f), np.asarray(bo, f)

    wqT = _tf32(np.ascontiguousarray(wq.T.reshape(2, 128, 256)))
    wkT = _tf32(np.ascontiguousarray(wk.T.reshape(2, 128, 256)))
    woT = _tf32(np.ascontiguousarray(wo.T.reshape(2, 128, 256)))
    wvT = wv.T  # (Cin, Cout)
    wvTa = np.zeros((256, 260), f)
    bva = np.zeros((1, 260), f)
    for h in range(NH):
        wvTa[:, h * 65:h * 65 + 64] = wvT[:, h * 64:(h + 1) * 64]
        bva[0, h * 65:h * 65 + 64] = bv[h * 64:(h + 1) * 64]
        bva[0, h * 65 + 64] = 1.0
    wvTa = _tf32(np.ascontiguousarray(wvTa.reshape(2, 128, 260)))
    bva = _tf32(bva)

    def two(v):
        return np.ascontiguousarray(np.asarray(v, f).reshape(2, 128).T)

    gmap = np.zeros((128, 16), f)
    for p in range(128):
        gmap[p, p // CPG] = 1.0
    gmapb = np.ascontiguousarray(gmap.T)

    base = dict(
        wqT=wqT, wkT=wkT, wvTa=wvTa, woT=woT, bva=bva,
        bq2=two(bq), bk2=two(bk), bo2=two(bo),
        gqA=two(gn_q_w), gqB=two(gn_q_b), gkA=two(gn_kv_w), gkB=two(gn_kv_b),
        gmap=gmap, gmapb=gmapb,
        ones64=np.ones((1, 64), f), ones128=np.ones((1, 128), f),
    )

    xr = x.reshape(B, 2, 128, N)
    cr = cond.reshape(B, 2, 128, N)
    in_maps = []
    for c in range(NCORES):
        b, nh = c // 2, c % 2
        m = dict(base)
        # roll x so this core's query half occupies cols [0:NHALF];
        # GN stats are permutation-invariant over N so this is safe.
        m["xb"] = np.ascontiguousarray(np.roll(xr[b], -nh * NHALF, axis=2))
        m["condb"] = np.ascontiguousarray(cr[b])
        in_maps.append(m)
    return in_maps


def kernel(x, cond, gn_q_w, gn_q_b, gn_kv_w, gn_kv_b,
           wq, bq, wk, bk, wv, bv, wo, bo):
    if "nc" not in _cache:
        _cache["nc"] = _build()
    nc = _cache["nc"]
    in_maps = _prep_inputs(x, cond, gn_q_w, gn_q_b, gn_kv_w, gn_kv_b,
                           wq, bq, wk, bk, wv, bv, wo, bo)
    res = run_bass_kernel_spmd(
        nc, in_maps, core_ids=list(range(NCORES)),
        trace=bool(os.environ.get("KERNEL_TRACE")))
    _cache["last"] = res
    y = np.empty((B, C, N), np.float32)
    for c in range(NCORES):
        b, nh = c // 2, c % 2
        o = np.asarray(res.results[c]["out"]).reshape(256, NHALF)
        y[b, :, nh * NHALF:(nh + 1) * NHALF] = o
    return y.reshape(B, C, H, W)
